# revision 7
# baseline (speedup 1.0000x reference)
"""GCN 2-layer message passing on 8 Trainium2 NeuronCores.

Strategy (per sharding hint): nodes sharded 8x12500; edges partitioned by
dst shard, sorted by (dst-block, src-chunk); per-edge features gathered from
a replicated bf16 node-feature table via SWDGE dma_gather (4 queues); the
segment-sum is computed per 128-dst block as sum_g S_g^T-style one-hot
matmuls accumulated in PSUM (out^T layout [feat, dst]); epilogue applies
deg^-1/2 scaling, bias, relu. Two NEFFs (table build, message passing),
each run for both layers, host gathers/redistributes between launches.
"""

import os
import numpy as np

import concourse.bass as bass
import concourse.tile as tile
from concourse import bacc, mybir
from concourse.bass_utils import run_bass_kernel_spmd

N = 100000
D = 128
NC = 8
SHARD = 12500          # nodes per core
NBLK = 98              # 128-dst blocks per core (12544 padded)
PADN = NBLK * 128      # 12544
CHUNK = 25000          # table chunk (int16 index range)
NCHUNK = 4
SPAN = 4               # dst blocks per gather span

_f32 = mybir.dt.float32
_bf16 = mybir.dt.bfloat16
_i16 = mybir.dt.int16

BENCH = bool(int(os.environ.get("KERNEL_BENCH", "0")))
EXEC_NS = []


def _bcast3(t, G, last_stride0):
    """AP [128, G, 128] over tile t: last dim stride 0 (from [128,G]) or
    middle dim stride 0 (from [128,128])."""
    ap = t[:].ap
    pstride = ap[0][0]
    if last_stride0:
        return bass.AP(tensor=t.tensor, offset=t[:].offset,
                       ap=[[pstride, 128], [ap[1][0], G], [0, 128]])
    return bass.AP(tensor=t.tensor, offset=t[:].offset,
                   ap=[[pstride, 128], [0, G], [ap[1][0], 128]])


def build_table_nc():
    """h_shard = (x_shard @ W) * rsqrt(deg) as bf16 rows [PADN, D]."""
    nc = bacc.Bacc("TRN2", target_bir_lowering=False, debug=False,
                   enable_asserts=False, num_devices=NC)
    xT = nc.dram_tensor("xT", [D, PADN], _f32, kind="ExternalInput").ap()
    W = nc.dram_tensor("W", [D, D], _f32, kind="ExternalInput").ap()
    degc = nc.dram_tensor("degc", [128, NBLK], _f32, kind="ExternalInput").ap()
    tabo = nc.dram_tensor("tabo", [PADN, D], _bf16, kind="ExternalOutput").ap()

    with tile.TileContext(nc) as tc:
        with tc.tile_pool(name="sing", bufs=1) as sing, \
             tc.tile_pool(name="work", bufs=3) as work, \
             tc.tile_pool(name="ps", bufs=2, space="PSUM") as ps:
            xt = sing.tile([128, PADN], _f32)
            nc.sync.dma_start(out=xt[:], in_=xT[:])
            w = sing.tile([128, D], _f32)
            nc.sync.dma_start(out=w[:], in_=W[:])
            dc = sing.tile([128, NBLK], _f32)
            nc.sync.dma_start(out=dc[:], in_=degc[:])
            dsq = sing.tile([128, NBLK], _f32)
            nc.scalar.activation(dsq[:], dc[:],
                                 mybir.ActivationFunctionType.Sqrt)
            dis = sing.tile([128, NBLK], _f32)
            nc.vector.reciprocal(dis[:], dsq[:])
            for j in range(NBLK):
                p = ps.tile([128, D], _f32, space="PSUM")
                nc.tensor.matmul(out=p[:], lhsT=xt[:, j * 128:(j + 1) * 128],
                                 rhs=w[:], start=True, stop=True)
                o = work.tile([128, D], _bf16)
                nc.vector.tensor_scalar_mul(o[:], p[:], dis[:, j:j + 1])
                nc.sync.dma_start(out=tabo[j * 128:(j + 1) * 128, :], in_=o[:])
    nc.compile()
    return nc


def build_mp_nc(layout):
    """Message passing for one layer given the per-core-uniform span layout.

    layout: list of spans; each span dict:
      G: stripes in span
      calls: list of (chunk, idx_col_off, n_idx, g_off)
      blocks: list of (b_abs, [stripe indices])
    idxw: total idx columns; Gtot: total stripes.
    """
    nc = bacc.Bacc("TRN2", target_bir_lowering=False, debug=False,
                   enable_asserts=False, num_devices=NC, num_swdge_queues=4)
    tab = nc.dram_tensor("tab", [N, D], _bf16, kind="ExternalInput").ap()
    idxt = nc.dram_tensor("idxt", [128, layout["idxw"]], _i16,
                          kind="ExternalInput").ap()
    dstl = nc.dram_tensor("dstl", [128, layout["Gtot"]], _bf16,
                          kind="ExternalInput").ap()
    degr = nc.dram_tensor("degr", [1, PADN], _f32, kind="ExternalInput").ap()
    bias = nc.dram_tensor("bias", [128, 1], _f32, kind="ExternalInput").ap()
    outT = nc.dram_tensor("outT", [D, PADN], _f32, kind="ExternalOutput").ap()

    qi = 0
    with tile.TileContext(nc) as tc:
        with tc.tile_pool(name="sing", bufs=1) as sing, \
             tc.tile_pool(name="mp", bufs=3) as mp, \
             tc.tile_pool(name="sp", bufs=2) as sp, \
             tc.tile_pool(name="ip", bufs=3) as ip, \
             tc.tile_pool(name="dp", bufs=2) as dp, \
             tc.tile_pool(name="ep", bufs=4) as ep, \
             tc.tile_pool(name="ps", bufs=2, space="PSUM") as ps:
            dis = sing.tile([128, PADN], _f32)
            with tc.tile_pool(name="dtmp", bufs=2) as dtmp:
                DC = PADN // 8
                for jc in range(8):
                    dtc = dtmp.tile([128, DC], _f32, tag="dtc")
                    nc.gpsimd.dma_start(
                        out=dtc[:],
                        in_=bass.AP(tensor=degr.tensor,
                                    offset=degr.offset + jc * DC,
                                    ap=[[0, 128], [1, DC]]))
                    dsq = dtmp.tile([128, DC], _f32, tag="dsq")
                    nc.scalar.activation(dsq[:], dtc[:],
                                         mybir.ActivationFunctionType.Sqrt)
                    nc.vector.reciprocal(dis[:, jc * DC:(jc + 1) * DC], dsq[:])
            bs = sing.tile([128, 1], _f32)
            nc.sync.dma_start(out=bs[:], in_=bias[:])
            iota_i = sing.tile([128, 128], _i16)
            nc.gpsimd.iota(iota_i[:], pattern=[[1, 128]], base=0,
                           channel_multiplier=0)
            iota_b = sing.tile([128, 128], _bf16)
            nc.vector.tensor_copy(iota_b[:], iota_i[:])

            for span in layout["spans"]:
                G = span["G"]
                it = ip.tile([128, 8 * G], _i16, tag="it")
                nc.sync.dma_start(
                    out=it[:, :],
                    in_=idxt[:, span["idx_off"]:span["idx_off"] + 8 * G])
                m = mp.tile([128, G, 128], _bf16, tag="m")
                for (c, coff, n_idx, g_off) in span["calls"]:
                    nc.gpsimd.dma_gather(
                        out_ap=m[:, g_off:g_off + n_idx // 128, :],
                        in_ap=tab[c * CHUNK:(c + 1) * CHUNK, :],
                        idxs_ap=it[:, coff:coff + n_idx // 16],
                        num_idxs=n_idx,
                        num_idxs_reg=n_idx,
                        elem_size=D,
                        single_packet=False,
                        queue_num=qi % 4,
                    )
                    qi += 1
                dt = dp.tile([128, G], _bf16, tag="dt")
                nc.sync.dma_start(
                    out=dt[:],
                    in_=dstl[:, span["g_off"]:span["g_off"] + G])
                s = sp.tile([128, G, 128], _bf16, tag="s")
                nc.vector.tensor_tensor(
                    out=s[:], in0=_bcast3(dt, G, True),
                    in1=_bcast3(iota_b, G, False),
                    op=mybir.AluOpType.is_equal)
                for (b, stripes) in span["blocks"]:
                    p = ps.tile([128, 128], _f32, space="PSUM")
                    for k, g in enumerate(stripes):
                        nc.tensor.matmul(out=p[:], lhsT=m[:, g, :],
                                         rhs=s[:, g, :], start=(k == 0),
                                         stop=(k == len(stripes) - 1))
                    t1 = ep.tile([128, 128], _f32, tag="t1")
                    nc.vector.tensor_tensor(
                        out=t1[:], in0=p[:],
                        in1=dis[:, b * 128:(b + 1) * 128],
                        op=mybir.AluOpType.mult)
                    t2 = ep.tile([128, 128], _f32, tag="t2")
                    nc.scalar.activation(t2[:], t1[:],
                                         mybir.ActivationFunctionType.Relu,
                                         bias=bs[:], scale=1.0)
                    nc.sync.dma_start(out=outT[:, b * 128:(b + 1) * 128],
                                      in_=t2[:])
    nc.compile()
    return nc


def prep_edges(edge_index):
    """Per-core edge layout. Returns per-core input arrays + shared layout.

    The layout (span/call/block structure) must be IDENTICAL across cores
    (one SPMD program), so group counts per (block, chunk) are padded to
    the per-(b,c) max over cores.
    """
    src = np.concatenate([edge_index[0], np.arange(N, dtype=np.int64)])
    dst = np.concatenate([edge_index[1], np.arange(N, dtype=np.int64)])
    deg = np.bincount(dst, minlength=N).astype(np.float32)

    per_core = []
    cnts = np.zeros((NC, NBLK, NCHUNK), dtype=np.int64)
    for c in range(NC):
        sel = (dst >= c * SHARD) & (dst < (c + 1) * SHARD)
        s_ = src[sel]
        dl = dst[sel] - c * SHARD
        blk = dl // 128
        chk = s_ // CHUNK
        order = np.lexsort((dl, chk, blk))
        s_, dl, blk, chk = s_[order], dl[order], blk[order], chk[order]
        per_core.append((s_, dl, blk, chk))
        np.add.at(cnts[c], (blk, chk), 1)

    gmax = (cnts.max(axis=0) + 127) // 128  # [NBLK, NCHUNK] groups per cell

    # shared layout
    spans = []
    g_abs = 0
    idx_off = 0
    for s0 in range(0, NBLK, SPAN):
        bl = list(range(s0, min(s0 + SPAN, NBLK)))
        calls = []
        g_in_span = 0
        blocks = {b: [] for b in bl}
        for c in range(NCHUNK):
            n_grp = int(sum(gmax[b, c] for b in bl))
            if n_grp == 0:
                continue
            calls.append((c, 8 * g_in_span, n_grp * 128, g_in_span))
            for b in bl:
                for _ in range(int(gmax[b, c])):
                    blocks[b].append(g_in_span)
                    g_in_span += 1
        G = g_in_span
        spans.append({
            "G": G, "calls": calls, "g_off": g_abs, "idx_off": idx_off,
            "blocks": [(b, blocks[b]) for b in bl],
        })
        g_abs += G
        idx_off += 8 * G
    layout = {"spans": spans, "Gtot": g_abs, "idxw": idx_off}

    # per-core data tensors
    datas = []
    for c in range(NC):
        s_, dl, blk, chk = per_core[c]
        idx_arr = np.zeros((128, layout["idxw"]), dtype=np.int16)
        dstl_arr = np.full((128, layout["Gtot"]), 255.0, dtype=np.float32)
        # cell start offsets in the sorted per-core stream
        cell_n = np.zeros((NBLK, NCHUNK), dtype=np.int64)
        np.add.at(cell_n, (blk, chk), 1)
        cell_start = np.zeros(NBLK * NCHUNK + 1, dtype=np.int64)
        np.cumsum(cell_n.ravel(), out=cell_start[1:])
        for span in spans:
            for (b, stripes) in span["blocks"]:
                si = 0
                for ch in range(NCHUNK):
                    st = cell_start[b * NCHUNK + ch]
                    n_real = int(cell_n[b, ch])
                    n_pad_grp = int(gmax[b, ch])
                    loc_i = np.zeros(n_pad_grp * 128, dtype=np.int16)
                    loc_d = np.full(n_pad_grp * 128, 255.0, dtype=np.float32)
                    loc_i[:n_real] = (s_[st:st + n_real] % CHUNK).astype(np.int16)
                    loc_d[:n_real] = (dl[st:st + n_real] % 128).astype(np.float32)
                    for k in range(n_pad_grp):
                        g = span["g_off"] + stripes[si]
                        eg = loc_i[k * 128:(k + 1) * 128]
                        dstl_arr[:, g] = loc_d[k * 128:(k + 1) * 128]
                        # idx wrap: call-local idx i -> (ch i%16, slot i//16)
                        gi = stripes[si]  # span-local stripe
                        col0 = span["idx_off"] + gi * 8
                        wr = eg.reshape(8, 16).T  # [16, 8]
                        idx_arr[:, col0:col0 + 8] = np.tile(wr, (8, 1))
                        si += 1
        datas.append({"idxt": idx_arr,
                      "dstl": dstl_arr.astype(np.dtype("bfloat16"))})
    return deg, layout, datas


_CACHE = {}


def kernel(x, edge_index, W1, b1, W2, b2):
    x = np.asarray(x)
    edge_index = np.asarray(edge_index).astype(np.int64)
    W1, b1 = np.asarray(W1), np.asarray(b1)
    W2, b2 = np.asarray(W2), np.asarray(b2)

    deg, layout, datas = prep_edges(edge_index)

    if "tab" not in _CACHE:
        _CACHE["tab"] = build_table_nc()
    if "mp" not in _CACHE:
        _CACHE["mp"] = build_mp_nc(layout)
    tab_nc, mp_nc = _CACHE["tab"], _CACHE["mp"]

    core_ids = list(range(NC))
    degc = []
    degr = []
    for c in range(NC):
        dshard = np.ones(PADN, dtype=np.float32)
        dshard[:SHARD] = deg[c * SHARD:(c + 1) * SHARD]
        degc.append(dshard.reshape(NBLK, 128).T.copy())
        degr.append(dshard.reshape(1, PADN))

    def run_table(xT_shards, W):
        in_maps = [{"xT": xT_shards[c], "W": W, "degc": degc[c]}
                   for c in core_ids]
        print("launch: table", flush=True)
        res = run_bass_kernel_spmd(tab_nc, in_maps, core_ids, trace=BENCH)
        if BENCH:
            EXEC_NS.append(res.exec_time_ns)
        shards = [np.asarray(res.results[c]["tabo"])[:SHARD] for c in core_ids]
        return np.concatenate(shards, axis=0)  # [N, D] bf16

    def run_mp(tab_full, b):
        bcol = np.tile(b.reshape(128, 1), (1, 1)).astype(np.float32)
        in_maps = [{"tab": tab_full, "idxt": datas[c]["idxt"],
                    "dstl": datas[c]["dstl"], "degr": degr[c], "bias": bcol}
                   for c in core_ids]
        print("launch: mp", flush=True)
        res = run_bass_kernel_spmd(mp_nc, in_maps, core_ids, trace=BENCH)
        if BENCH:
            EXEC_NS.append(res.exec_time_ns)
        return [np.asarray(res.results[c]["outT"]) for c in core_ids]

    # layer 1
    xT = []
    for c in range(NC):
        t = np.zeros((D, PADN), dtype=np.float32)
        t[:, :SHARD] = x[c * SHARD:(c + 1) * SHARD].T
        xT.append(t)
    tab1 = run_table(xT, np.ascontiguousarray(W1))
    o1 = run_mp(tab1, b1)
    # layer 2
    tab2 = run_table(o1, np.ascontiguousarray(W2))
    o2 = run_mp(tab2, b2)

    out = np.empty((N, D), dtype=np.float32)
    for c in range(NC):
        out[c * SHARD:(c + 1) * SHARD] = o2[c][:, :SHARD].T
    return out
